# revision 5
# baseline (speedup 1.0000x reference)
"""Trainium2 Bass kernel for nn_Conv2DLinalgRMSNorm.

Math: out = RMSNormEps(x @ (sum_l conv_w[l])^T / 20) * norm_w
  where RMSNormEps(v) = v / sqrt(sum_h v^2 + eps*H) * sqrt(H)

Strategy (8 NeuronCores, no cross-core collectives), all GEMM I/O in bf16:
  Host prep (layout/dtype only): conv_w and x^T are cast to bf16; x is
  pre-transposed to [h_in, tok] so L2 needs no on-chip transposes.
  Launch 1 (weight prep, sharded over 128-row output-channel slices):
    core i reads conv_w[:, i*128:(i+1)*128, :] in bf16 (5.2 MB, 2 KiB DMA
    lines on 2 queues), accumulates 20 layers via two running-sum chains
    (evens on DVE, odds on GpSimd), PE transpose-accumulates the two
    chains per 128-block and writes its [1024, 128] slice of W_sum^T bf16.
  Launch 2 (token-parallel GEMM + norm):
    core i loads full W^T (2 MB) + its x^T token slice (2 MB), runs the
    [1024 tok x 1024 x 1024] GEMM with h_in-chunk-outer ordering over 4
    concurrent PSUM tiles (all 8 banks) so the PE streams while DMA fills,
    then fuses LinalgRMSNorm on ACT/DVE.  The 1/20 scaling folds into the
    rsqrt bias: out = y * 32 * rsqrt(sum y^2 + 400*eps*H) * norm_w.
"""
import numpy as np
import ml_dtypes

import concourse.bass as bass
import concourse.mybir as mybir
from concourse.tile import TileContext
from concourse import bass_utils

dt = mybir.dt
BF16 = ml_dtypes.bfloat16
P = 128
H = 1024
NL = 20
B, S = 2, 4096
TOK = B * S            # 8192
NCORES = 8
TPC = TOK // NCORES    # 1024 tokens per core
NCH = H // P           # 8 h_in chunks of 128
NT = TPC // P          # 8 token tiles per core
EPS = 1e-6
SSQ_BIAS = float(NL * NL * EPS * H)   # 0.4096

_ctr = [0]


def _legalize_waits(nc):
    """This walrus build accepts 1 sync wait per instruction (2 on
    EventSemaphore); split excess waits into standalone waits."""
    def fix_block(blk):
        insts = list(blk.instructions)
        out = []
        changed = False
        for inst in insts:
            si = inst.sync_info
            waits = list(si.on_wait) if si and si.on_wait else []
            cap = 2 if isinstance(inst, mybir.InstEventSemaphore) else 1
            if len(waits) > cap:
                changed = True
                keep = waits[:cap]
                extra = waits[cap:]
                for i in range(0, len(extra), 2):
                    chunk = extra[i:i + 2]
                    _ctr[0] += 1
                    ev = mybir.InstEventSemaphore(
                        name=f"I-waitfix-{_ctr[0]}",
                        engine=inst.engine,
                        ins=[],
                        outs=[],
                        sync_info=mybir.SyncInfo(on_wait=chunk, on_update=[]),
                    )
                    out.append(ev)
                si.on_wait = keep
            out.append(inst)
        if changed:
            blk.instructions = out
        for sub in getattr(blk, "blocks", None) or []:
            fix_block(sub)

    for fn in nc.m.functions:
        for blk in fn.blocks:
            fix_block(blk)


def _make_identity(nc, identity):
    nc.gpsimd.memset(identity, 0.0)
    nc.gpsimd.affine_select(
        out=identity,
        in_=identity,
        compare_op=mybir.AluOpType.not_equal,
        fill=1.0,
        base=0,
        pattern=[[-1, identity.shape[0]]],
        channel_multiplier=1,
    )


def build_l1():
    """Weight prep: conv slice [20, 128, 1024] bf16 -> wt piece [1024, 128] bf16.

    Per-layer contiguous loads (2 KiB lines) alternate between the Sync and
    Activation DMA queues; two running-sum chains accumulate in f32 (even
    layers on DVE, odd on GpSimd); the PE transpose-accumulates both chains
    per 128-block and ACT rounds PSUM to bf16 for the output slice.
    """
    nc = bass.Bass('TRN2', target_bir_lowering=False, debug=False)
    cw = nc.dram_tensor("cw", [NL, P, H], dt.bfloat16, kind="ExternalInput")
    wtp = nc.dram_tensor("wtp", [H, P], dt.bfloat16, kind="ExternalOutput")
    with TileContext(nc) as tc:
        with (
            tc.tile_pool(name="ld", bufs=1) as ld,
            tc.tile_pool(name="acc", bufs=1) as accp,
            tc.tile_pool(name="out", bufs=4) as outp,
            tc.tile_pool(name="psum", bufs=4, space="PSUM") as psum,
            tc.tile_pool(name="psumw", bufs=1, space="PSUM") as psumw,
        ):
            ident = accp.tile([P, P], dt.float32, tag="ident")
            _make_identity(nc, ident[:])
            # PE warm-up: release the HAM clock gate while DMA streams in
            wu = psumw.tile([P, P], dt.float32, tag="wu")
            for _ in range(14):
                nc.tensor.matmul(wu[:], ident[:], ident[:], is_transpose=True,
                                 start=True, stop=True)

            t = ld.tile([P, NL, H], dt.bfloat16, tag="t")
            cwr = cw.rearrange("l p h -> p l h")
            qs = [nc.sync, nc.scalar]
            for l in range(NL):
                qs[l % 2].dma_start(t[:, l, :], cwr[:, l, :])

            # running sums: even layers on DVE, odd layers on GpSimd
            se = [accp.tile([P, H], dt.float32, tag=f"se{i}", name=f"se{i}") for i in range(2)]
            so = [accp.tile([P, H], dt.float32, tag=f"so{i}", name=f"so{i}") for i in range(2)]
            nc.vector.tensor_add(se[0][:], t[:, 0, :], t[:, 2, :])
            ce = 0
            for k in range(2, 10):
                nc.vector.tensor_add(se[1 - ce][:], se[ce][:], t[:, 2 * k, :])
                ce = 1 - ce
            nc.gpsimd.tensor_add(so[0][:], t[:, 1, :], t[:, 3, :])
            co = 0
            for k in range(2, 10):
                nc.gpsimd.tensor_add(so[1 - co][:], so[co][:], t[:, 2 * k + 1, :])
                co = 1 - co
            sef, sof = se[ce], so[co]

            # late re-warm: dummy transposes gated on the last-loaded layer
            # keep the PE clock up for the real transpose tail
            identb = accp.tile([P, P], dt.bfloat16, tag="identb")
            _make_identity(nc, identb[:])
            wub = psumw.tile([P, P], dt.bfloat16, tag="wub")
            for r in range(6):
                nc.tensor.matmul(wub[:], t[:, NL - 1, bass.ds(r * P, P)],
                                 identb[:], is_transpose=True,
                                 start=True, stop=True)

            # W^T slice: transpose-accumulate the two chains per 128-block
            for c in range(NCH):
                csl = bass.ds(c * P, P)
                pt = psum.tile([P, P], dt.float32, tag="pt")
                nc.tensor.matmul(pt[:], sef[:, csl], ident[:],
                                 is_transpose=True, start=True, stop=False)
                nc.tensor.matmul(pt[:], sof[:, csl], ident[:],
                                 is_transpose=True, start=False, stop=True)
                wb = outp.tile([P, P], dt.bfloat16, tag="wb")
                nc.scalar.activation(wb[:], pt[:],
                                     mybir.ActivationFunctionType.Copy)
                qs[c % 2].dma_start(wtp[csl, :], wb[:])
    _legalize_waits(nc)
    return nc


def build_l2():
    """Token-shard GEMM + LinalgRMSNorm, all-bf16 I/O, no on-chip transposes.

    x^T slice [1024 (h_in), 1024 (tok)] and W^T [1024, 1024] stream in as 8
    h_in chunks each on separate queues; the GEMM runs h_in-chunk-outer over
    4 concurrent [128, 1024] PSUM tiles (2 groups of 4 token tiles) so the
    PE consumes chunks as they land.  Norm fuses on ACT/DVE per token tile.
    """
    nc = bass.Bass('TRN2', target_bir_lowering=False, debug=False)
    xt = nc.dram_tensor("xt", [H, TPC], dt.bfloat16, kind="ExternalInput")
    wt = nc.dram_tensor("wt", [H, H], dt.bfloat16, kind="ExternalInput")
    nw = nc.dram_tensor("nw", [H], dt.float32, kind="ExternalInput")
    y = nc.dram_tensor("y", [TPC, H], dt.bfloat16, kind="ExternalOutput")
    NG = 2               # psum groups
    GT = NT // NG        # 4 token tiles per group
    with TileContext(nc) as tc:
        with (
            tc.tile_pool(name="w", bufs=1) as wp,
            tc.tile_pool(name="sq", bufs=2) as sqp,
            tc.tile_pool(name="yout", bufs=3) as yp,
            tc.tile_pool(name="stat", bufs=4) as stat,
            tc.tile_pool(name="psum", bufs=1, space="PSUM") as psum,
        ):
            xt_sb = wp.tile([P, NCH, TPC], dt.bfloat16, tag="xt_sb")
            wt_sb = wp.tile([P, NCH, H], dt.bfloat16, tag="wt_sb")
            xt_r = xt.rearrange("(c p) t -> p c t", p=P)
            wt_r = wt.rearrange("(c p) o -> p c o", p=P)
            for hc in range(NCH):
                nc.sync.dma_start(xt_sb[:, hc, :], xt_r[:, hc, :])
                nc.scalar.dma_start(wt_sb[:, hc, :], wt_r[:, hc, :])
            nwb = wp.tile([P, H], dt.float32, tag="nwb")
            nc.gpsimd.dma_start(nwb[:], nw[None, :].partition_broadcast(P))
            ident = wp.tile([P, P], dt.float32, tag="ident")
            _make_identity(nc, ident[:])

            pts = [psum.tile([P, H], dt.float32, tag=f"pt{i}", name=f"pt{i}") for i in range(GT)]
            # PE warm-up into a psum half that hc=0 later resets (start=True)
            for _ in range(12):
                nc.tensor.matmul(pts[0][:, bass.ds(0, P)], ident[:], ident[:],
                                 is_transpose=True, start=True, stop=True)

            for g in range(NG):
                for hc in range(NCH):
                    for i in range(GT):
                        tt = g * GT + i
                        lhs = xt_sb[:, hc, bass.ds(tt * P, P)]
                        for oh in range(2):
                            osl = bass.ds(oh * 512, 512)
                            nc.tensor.matmul(
                                pts[i][:, osl], lhs, wt_sb[:, hc, osl],
                                start=(hc == 0), stop=(hc == NCH - 1),
                            )
                for i in range(GT):
                    tt = g * GT + i
                    pt = pts[i]
                    sq = sqp.tile([P, H], dt.bfloat16, tag="sq")
                    v = stat.tile([P, 1], dt.float32, tag="v")
                    nc.scalar.activation(
                        sq[:], pt[:], mybir.ActivationFunctionType.Square,
                        accum_out=v[:],
                    )
                    vb = stat.tile([P, 1], dt.float32, tag="vb")
                    nc.vector.tensor_scalar(
                        vb[:], v[:], SSQ_BIAS, None, mybir.AluOpType.add,
                    )
                    rv = stat.tile([P, 1], dt.float32, tag="rv")
                    nc.vector.reciprocal(rv[:], vb[:])
                    s = stat.tile([P, 1], dt.float32, tag="s")
                    nc.scalar.activation(
                        s[:], rv[:], mybir.ActivationFunctionType.Sqrt,
                        scale=float(H),
                    )
                    ysb = yp.tile([P, H], dt.bfloat16, tag="ysb")
                    nc.vector.scalar_tensor_tensor(
                        ysb[:], pt[:], s[:], nwb[:],
                        op0=mybir.AluOpType.mult, op1=mybir.AluOpType.mult,
                    )
                    nc.sync.dma_start(y[bass.ds(tt * P, P), :], ysb[:])
    _legalize_waits(nc)
    return nc


_CACHE = {}


def _get(name, builder):
    if name not in _CACHE:
        _CACHE[name] = builder()
    return _CACHE[name]


def prep_l1_inputs(conv_w):
    cw_bf = np.asarray(conv_w, dtype=np.float32).astype(BF16)
    return [
        {"cw": np.ascontiguousarray(cw_bf[:, i * P:(i + 1) * P, :])}
        for i in range(NCORES)
    ]


def prep_l2_inputs(hidden_states, wt_full, norm_w):
    x_flat = np.asarray(hidden_states, dtype=np.float32).reshape(TOK, H)
    xt_bf = np.ascontiguousarray(x_flat.T.astype(BF16))   # [h_in, tok]
    nw = np.asarray(norm_w, dtype=np.float32)
    return [
        {
            "xt": np.ascontiguousarray(xt_bf[:, i * TPC:(i + 1) * TPC]),
            "wt": wt_full,
            "nw": nw,
        }
        for i in range(NCORES)
    ]


def kernel(hidden_states, conv_w, norm_w):
    in_dtype = hidden_states.dtype
    core_ids = list(range(NCORES))

    # Launch 1: weight prep
    nc1 = _get("l1", build_l1)
    res1 = bass_utils.run_bass_kernel_spmd(nc1, prep_l1_inputs(conv_w), core_ids)
    wt_full = np.ascontiguousarray(
        np.concatenate([res1.results[i]["wtp"] for i in range(NCORES)], axis=1)
    )

    # Launch 2: GEMM + norm over token shards
    nc2 = _get("l2", build_l2)
    in2 = prep_l2_inputs(hidden_states, wt_full, norm_w)
    res2 = bass_utils.run_bass_kernel_spmd(nc2, in2, core_ids)
    y = np.concatenate([res2.results[i]["y"] for i in range(NCORES)], axis=0)
    return y.astype(np.float32).reshape(B, S, H).astype(in_dtype, copy=False)


# revision 12
# speedup vs baseline: 1.4648x; 1.4648x over previous
"""Trainium2 Bass kernel for nn_Conv2DLinalgRMSNorm.

Math: out = RMSNormEps(x @ (sum_l conv_w[l])^T / 20) * norm_w
  where RMSNormEps(v) = v / sqrt(sum_h v^2 + eps*H) * sqrt(H)

Strategy (8 NeuronCores, no cross-core collectives), all GEMM I/O in bf16:
  Host prep (layout/dtype only): conv_w and x^T are cast to bf16; x is
  pre-transposed to [h_in, tok] so L2 needs no on-chip transposes.
  Launch 1 (weight prep, sharded over 128-row output-channel slices):
    core i reads conv_w[:, i*128:(i+1)*128, :] in bf16 (5.2 MB, 2 KiB DMA
    lines on 2 queues), accumulates 20 layers via two running-sum chains
    (evens on DVE, odds on GpSimd), PE transpose-accumulates the two
    chains per 128-block and writes its [1024, 128] slice of W_sum^T bf16.
  Launch 2 (token-parallel GEMM + norm):
    core i loads full W^T (2 MB) + its x^T token slice (2 MB), runs the
    [1024 tok x 1024 x 1024] GEMM with h_in-chunk-outer ordering over 4
    concurrent PSUM tiles (all 8 banks) so the PE streams while DMA fills,
    then fuses LinalgRMSNorm on ACT/DVE.  The 1/20 scaling folds into the
    rsqrt bias: out = y * 32 * rsqrt(sum y^2 + 400*eps*H) * norm_w.
"""
import numpy as np
import ml_dtypes

import concourse.bass as bass
import concourse.mybir as mybir
from concourse.tile import TileContext
from concourse import bass_utils

dt = mybir.dt
BF16 = ml_dtypes.bfloat16
P = 128
H = 1024
NL = 20
B, S = 2, 4096
TOK = B * S            # 8192
NCORES = 8
TPC = TOK // NCORES    # 1024 tokens per core
NCH = H // P           # 8 h_in chunks of 128
NT = TPC // P          # 8 token tiles per core
EPS = 1e-6
SSQ_BIAS = float(NL * NL * EPS * H)   # 0.4096

_ctr = [0]


def _legalize_waits(nc):
    """This walrus build accepts 1 sync wait per instruction (2 on
    EventSemaphore); split excess waits into standalone waits."""
    def fix_block(blk):
        insts = list(blk.instructions)
        out = []
        changed = False
        for inst in insts:
            si = inst.sync_info
            waits = list(si.on_wait) if si and si.on_wait else []
            cap = 2 if isinstance(inst, mybir.InstEventSemaphore) else 1
            if len(waits) > cap:
                changed = True
                keep = waits[:cap]
                extra = waits[cap:]
                for i in range(0, len(extra), 2):
                    chunk = extra[i:i + 2]
                    _ctr[0] += 1
                    ev = mybir.InstEventSemaphore(
                        name=f"I-waitfix-{_ctr[0]}",
                        engine=inst.engine,
                        ins=[],
                        outs=[],
                        sync_info=mybir.SyncInfo(on_wait=chunk, on_update=[]),
                    )
                    out.append(ev)
                si.on_wait = keep
            out.append(inst)
        if changed:
            blk.instructions = out
        for sub in getattr(blk, "blocks", None) or []:
            fix_block(sub)

    for fn in nc.m.functions:
        for blk in fn.blocks:
            fix_block(blk)


def _make_identity(nc, identity):
    nc.gpsimd.memset(identity, 0.0)
    nc.gpsimd.affine_select(
        out=identity,
        in_=identity,
        compare_op=mybir.AluOpType.not_equal,
        fill=1.0,
        base=0,
        pattern=[[-1, identity.shape[0]]],
        channel_multiplier=1,
    )


def build_l1():
    """Weight prep: conv slice [20, 128, 1024] bf16 -> wt piece [1024, 128] bf16.

    Per-layer contiguous loads (2 KiB lines) round-robin over the Sync,
    Activation and GpSimd DMA queues; two all-bf16 parity running-sum chains
    on DVE (2-byte operands hit the DVE 2x mode); the PE transpose-
    accumulates both chains per 128-block in bf16 PSUM and ACT copies the
    result out.
    """
    nc = bass.Bass('TRN2', target_bir_lowering=False, debug=False)
    cw = nc.dram_tensor("cw", [NL, P, H], dt.bfloat16, kind="ExternalInput")
    wtp = nc.dram_tensor("wtp", [H, P], dt.bfloat16, kind="ExternalOutput")
    NPE = 6               # layers 14..19 accumulate directly on the PE
    NCHAIN = NL - NPE     # layers 0..13 via two DVE parity chains
    with TileContext(nc) as tc:
        with (
            tc.tile_pool(name="ld", bufs=1) as ld,
            tc.tile_pool(name="acc", bufs=1) as accp,
            tc.tile_pool(name="out", bufs=4) as outp,
            tc.tile_pool(name="psum", bufs=1, space="PSUM") as psum,
        ):
            identb = accp.tile([P, P], dt.bfloat16, tag="identb")
            _make_identity(nc, identb[:])

            # one f32 psum block per 128-column slice of this core's W rows:
            # out = block^T @ I accumulates the transpose of every operand
            pts = [psum.tile([P, P], dt.float32, tag=f"pt{c}", name=f"pt{c}")
                   for c in range(NCH)]
            # PE warm-up into pts[0], which the real accumulation later
            # resets via start=True
            for _ in range(14):
                nc.tensor.matmul(pts[0][:], identb[:], identb[:],
                                 start=True, stop=True)

            t = ld.tile([P, NL, H], dt.bfloat16, tag="t")
            cwr = cw.rearrange("l p h -> p l h")
            # chain layers on the two fast HWDGE queues; PE layers on the
            # gpsimd SWDGE queue (latency-tolerant, ~1us per issue)
            for k in range(NCHAIN // 2):
                nc.sync.dma_start(t[:, 2 * k, :], cwr[:, 2 * k, :])
                nc.scalar.dma_start(t[:, 2 * k + 1, :], cwr[:, 2 * k + 1, :])
            for l in range(NCHAIN, NL):
                nc.gpsimd.dma_start(t[:, l, :], cwr[:, l, :])

            # all-bf16 parity running sums, both on DVE (2x 16-bit mode)
            se = [accp.tile([P, H], dt.bfloat16, tag=f"se{i}", name=f"se{i}") for i in range(2)]
            so = [accp.tile([P, H], dt.bfloat16, tag=f"so{i}", name=f"so{i}") for i in range(2)]
            nc.vector.tensor_add(se[0][:], t[:, 0, :], t[:, 2, :])
            nc.vector.tensor_add(so[0][:], t[:, 1, :], t[:, 3, :])
            ce = co = 0
            for k in range(2, NCHAIN // 2):
                nc.vector.tensor_add(se[1 - ce][:], se[ce][:], t[:, 2 * k, :])
                ce = 1 - ce
                nc.vector.tensor_add(so[1 - co][:], so[co][:], t[:, 2 * k + 1, :])
                co = 1 - co
            sef, sof = se[ce], so[co]

            # PE layers accumulate into psum as they land (layer-outer so
            # each arriving layer immediately feeds 8 short matmuls)
            for l in range(NCHAIN, NL):
                for c in range(NCH):
                    csl = bass.ds(c * P, P)
                    nc.tensor.matmul(pts[c][:], t[:, l, csl], identb[:],
                                     start=(l == NCHAIN), stop=False)
            # the two chain transposes close each block
            for c in range(NCH):
                csl = bass.ds(c * P, P)
                nc.tensor.matmul(pts[c][:], sef[:, csl], identb[:],
                                 start=False, stop=False)
                nc.tensor.matmul(pts[c][:], sof[:, csl], identb[:],
                                 start=False, stop=True)
                wb = outp.tile([P, P], dt.bfloat16, tag="wb")
                nc.scalar.activation(wb[:], pts[c][:],
                                     mybir.ActivationFunctionType.Copy)
                (nc.sync if c % 2 == 0 else nc.scalar).dma_start(wtp[csl, :], wb[:])
    _legalize_waits(nc)
    return nc


def build_l2():
    """Token-shard GEMM + LinalgRMSNorm, all-bf16 I/O, no on-chip transposes.

    x^T slice [1024 (h_in), 1024 (tok)] and W^T [1024, 1024] stream in as 8
    h_in chunks each on separate queues; the GEMM runs h_in-chunk-outer over
    4 concurrent [128, 1024] PSUM tiles (2 groups of 4 token tiles) so the
    PE consumes chunks as they land.  Norm fuses on ACT/DVE per token tile.
    """
    nc = bass.Bass('TRN2', target_bir_lowering=False, debug=False)
    xt = nc.dram_tensor("xt", [H, TPC], dt.bfloat16, kind="ExternalInput")
    wt = nc.dram_tensor("wt", [H, H], dt.bfloat16, kind="ExternalInput")
    nw = nc.dram_tensor("nw", [H], dt.float32, kind="ExternalInput")
    y = nc.dram_tensor("y", [TPC, H], dt.bfloat16, kind="ExternalOutput")
    NG = 2               # psum groups
    GT = NT // NG        # 4 token tiles per group
    with TileContext(nc) as tc:
        with (
            tc.tile_pool(name="w", bufs=1) as wp,
            tc.tile_pool(name="sq", bufs=2) as sqp,
            tc.tile_pool(name="yout", bufs=3) as yp,
            tc.tile_pool(name="stat", bufs=4) as stat,
            tc.tile_pool(name="psum", bufs=1, space="PSUM") as psum,
        ):
            xt_sb = wp.tile([P, NCH, TPC], dt.bfloat16, tag="xt_sb")
            wt_sb = wp.tile([P, NCH, H], dt.bfloat16, tag="wt_sb")
            xt_r = xt.rearrange("(c p) t -> p c t", p=P)
            wt_r = wt.rearrange("(c p) o -> p c o", p=P)
            # identity first so the PE warm-up unblocks early; gpsimd DMA
            # issues follow it in that engine's program order
            ident = wp.tile([P, P], dt.float32, tag="ident")
            _make_identity(nc, ident[:])
            # 3-queue fill ordered by first-use: wt chunks 0-5 on scalar,
            # xt 0-3 then wt 6-7 on sync, xt 4-7 on gpsimd
            for hc in range(6):
                nc.scalar.dma_start(wt_sb[:, hc, :], wt_r[:, hc, :])
            for hc in range(4):
                nc.sync.dma_start(xt_sb[:, hc, :], xt_r[:, hc, :])
            for hc in range(6, NCH):
                nc.sync.dma_start(wt_sb[:, hc, :], wt_r[:, hc, :])
            for hc in range(4, NCH):
                nc.gpsimd.dma_start(xt_sb[:, hc, :], xt_r[:, hc, :])
            nwb = wp.tile([P, H], dt.float32, tag="nwb")
            nc.gpsimd.dma_start(nwb[:], nw[None, :].partition_broadcast(P))

            pts = [psum.tile([P, H], dt.float32, tag=f"pt{i}", name=f"pt{i}") for i in range(GT)]
            # PE warm-up into a psum half that hc=0 later resets (start=True)
            for _ in range(18):
                nc.tensor.matmul(pts[0][:, bass.ds(0, P)], ident[:], ident[:],
                                 is_transpose=True, start=True, stop=True)

            def norm_out(tt, pt):
                sq = sqp.tile([P, H], dt.bfloat16, tag="sq", name="sq")
                v = stat.tile([P, 1], dt.float32, tag="v", name="v")
                nc.scalar.activation(
                    sq[:], pt[:], mybir.ActivationFunctionType.Square,
                    accum_out=v[:],
                )
                vb = stat.tile([P, 1], dt.float32, tag="vb", name="vb")
                nc.vector.tensor_scalar(
                    vb[:], v[:], SSQ_BIAS, None, mybir.AluOpType.add,
                )
                rv = stat.tile([P, 1], dt.float32, tag="rv", name="rv")
                nc.vector.reciprocal(rv[:], vb[:])
                s = stat.tile([P, 1], dt.float32, tag="s", name="s")
                nc.scalar.activation(
                    s[:], rv[:], mybir.ActivationFunctionType.Sqrt,
                    scale=float(H),
                )
                ysb = yp.tile([P, H], dt.bfloat16, tag="ysb", name="ysb")
                nc.vector.scalar_tensor_tensor(
                    ysb[:], pt[:], s[:], nwb[:],
                    op0=mybir.AluOpType.mult, op1=mybir.AluOpType.mult,
                )
                nc.sync.dma_start(y[bass.ds(tt * P, P), :], ysb[:])

            # group 0 (tiles 0-2 on psum slots 0-2): hc-outer waves track
            # the DMA fill; slot 3 stays free so tile 3 starts stall-free
            for hc in range(NCH):
                for i in range(3):
                    lhs = xt_sb[:, hc, bass.ds(i * P, P)]
                    for oh in range(2):
                        osl = bass.ds(oh * 512, 512)
                        nc.tensor.matmul(
                            pts[i][:, osl], lhs, wt_sb[:, hc, osl],
                            start=(hc == 0), stop=(hc == NCH - 1),
                        )
            for i in range(3):
                norm_out(i, pts[i])
            # tiles 3-7 sequential on rotating slots [3, 0, 1, 2, 3]:
            # each slot's norm has ~3 tiles of slack before reuse
            for k, tt in enumerate(range(3, NT)):
                slot = (3 + k) % GT
                for hc in range(NCH):
                    lhs = xt_sb[:, hc, bass.ds(tt * P, P)]
                    for oh in range(2):
                        osl = bass.ds(oh * 512, 512)
                        nc.tensor.matmul(
                            pts[slot][:, osl], lhs, wt_sb[:, hc, osl],
                            start=(hc == 0), stop=(hc == NCH - 1),
                        )
                norm_out(tt, pts[slot])
    _legalize_waits(nc)
    return nc


_CACHE = {}


def _get(name, builder):
    if name not in _CACHE:
        _CACHE[name] = builder()
    return _CACHE[name]


def prep_l1_inputs(conv_w):
    cw_bf = np.asarray(conv_w, dtype=np.float32).astype(BF16)
    return [
        {"cw": np.ascontiguousarray(cw_bf[:, i * P:(i + 1) * P, :])}
        for i in range(NCORES)
    ]


def prep_l2_inputs(hidden_states, wt_full, norm_w):
    x_flat = np.asarray(hidden_states, dtype=np.float32).reshape(TOK, H)
    xt_bf = np.ascontiguousarray(x_flat.T.astype(BF16))   # [h_in, tok]
    nw = np.asarray(norm_w, dtype=np.float32)
    return [
        {
            "xt": np.ascontiguousarray(xt_bf[:, i * TPC:(i + 1) * TPC]),
            "wt": wt_full,
            "nw": nw,
        }
        for i in range(NCORES)
    ]


def kernel(hidden_states, conv_w, norm_w):
    in_dtype = hidden_states.dtype
    core_ids = list(range(NCORES))

    # Launch 1: weight prep
    nc1 = _get("l1", build_l1)
    res1 = bass_utils.run_bass_kernel_spmd(nc1, prep_l1_inputs(conv_w), core_ids)
    wt_full = np.ascontiguousarray(
        np.concatenate([res1.results[i]["wtp"] for i in range(NCORES)], axis=1)
    )

    # Launch 2: GEMM + norm over token shards
    nc2 = _get("l2", build_l2)
    in2 = prep_l2_inputs(hidden_states, wt_full, norm_w)
    res2 = bass_utils.run_bass_kernel_spmd(nc2, in2, core_ids)
    y = np.concatenate([res2.results[i]["y"] for i in range(NCORES)], axis=0)
    return y.astype(np.float32).reshape(B, S, H).astype(in_dtype, copy=False)
